# revision 27
# baseline (speedup 1.0000x reference)
"""Trainium2 Bass kernel for nn_ConvOffset2D_nonlocal2 (topk_masking).

Structure exploited:
  matrix[b,j,s,t] = a_ext[b,j,s] + a_int[b,j,t] + b_fus   (rank-1 + broadcast)
so
  out_pre[b,c,j,s] = (a_ext[s]+k)*S_x[c] + D[c],  S_x = sum_t x, D = sum_t a_int[t] x[:,t]
and the masked top-2 over t of matrix[...,s,:] is independent of s (row offset
does not change ranks): per (b,j) it is the top-3 of a_int with two special rows.

Data-parallel over B across 8 cores; sync-BN via two tiny in-kernel AllReduces.
"""

import numpy as np

import concourse.bass as bass
import concourse.mybir as mybir
import concourse.tile as tile
from concourse import bacc, library_config
from concourse.bass_utils import run_bass_kernel_spmd

dt = mybir.dt
Alu = mybir.AluOpType
Act = mybir.ActivationFunctionType

B, C, J, T = 16, 128, 25, 256
NCORES = 8
BL = B // NCORES            # 2 samples per core
NBJ = BL * J                # 50 (b,j) pairs per core
NLOC = BL * J * T           # 12800 free elements per core
NGLOB = float(B * J * T)    # 102400
EPS = 1e-5
NCHUNK = NLOC // 512        # 25 conv chunks of 512


def build_bass():
    nc = bacc.Bacc(None, target_bir_lowering=False)

    # ---- I/O ----
    x_d = nc.declare_dram_parameter("x", [C, NLOC], dt.float32, isOutput=False)
    wTi_d = nc.declare_dram_parameter("wTi", [C, C], dt.float32, isOutput=False)
    wTe_d = nc.declare_dram_parameter("wTe", [C, C], dt.float32, isOutput=False)
    bi_d = nc.declare_dram_parameter("bi", [C], dt.float32, isOutput=False)
    be_d = nc.declare_dram_parameter("be", [C], dt.float32, isOutput=False)
    wfi_d = nc.declare_dram_parameter("wfi", [C], dt.float32, isOutput=False)
    wfe_d = nc.declare_dram_parameter("wfe", [C], dt.float32, isOutput=False)
    bfus_d = nc.declare_dram_parameter("bfus", [C], dt.float32, isOutput=False)
    gi_d = nc.declare_dram_parameter("gi", [C], dt.float32, isOutput=False)
    bti_d = nc.declare_dram_parameter("bti", [C], dt.float32, isOutput=False)
    ge_d = nc.declare_dram_parameter("ge", [C], dt.float32, isOutput=False)
    bte_d = nc.declare_dram_parameter("bte", [C], dt.float32, isOutput=False)
    go_d = nc.declare_dram_parameter("go", [C], dt.float32, isOutput=False)
    bto_d = nc.declare_dram_parameter("bto", [C], dt.float32, isOutput=False)

    out_d = nc.declare_dram_parameter("out", [BL, C, J, T], dt.float32, isOutput=True)
    path_d = nc.declare_dram_parameter("path", [BL, C, J, T, 3], dt.float32, isOutput=True)
    idx3_d = nc.declare_dram_parameter("idx3", [BL, J, T, 3], dt.int32, isOutput=True)

    col = lambda d: d.rearrange("(c one) -> c one", one=1)

    with tile.TileContext(nc) as tc:
        from contextlib import ExitStack

        ctx = ExitStack()
        with ctx:
            consts = ctx.enter_context(tc.tile_pool(name="consts", bufs=1))
            dram = ctx.enter_context(tc.tile_pool(name="dram", bufs=1, space="DRAM"))
            small = ctx.enter_context(tc.tile_pool(name="small", bufs=1))

            # ---- constants to SBUF ----
            wTi = consts.tile([C, C], dt.float32)
            wTe = consts.tile([C, C], dt.float32)
            nc.sync.dma_start(out=wTi[:, :], in_=wTi_d[:, :])
            nc.sync.dma_start(out=wTe[:, :], in_=wTe_d[:, :])
            cols = {}
            for name, d in [("bi", bi_d), ("be", be_d), ("wfi", wfi_d), ("wfe", wfe_d),
                            ("bfus", bfus_d), ("gi", gi_d), ("bti", bti_d), ("ge", ge_d),
                            ("bte", bte_d), ("go", go_d), ("bto", bto_d)]:
                t_ = consts.tile([C, 1], dt.float32, name="c_" + name)
                nc.sync.dma_start(out=t_[:, :], in_=col(d))
                cols[name] = t_
            ones_r = consts.tile([1, C], dt.float32)
            nc.vector.memset(ones_r[:, :], 1.0)
            ones_f = consts.tile([C, C], dt.float32)
            nc.vector.memset(ones_f[:, :], 1.0)

            # ---- x load: (c, b j t) ----
            xpool = ctx.enter_context(tc.tile_pool(name="xpool", bufs=1))
            x_sb = xpool.tile([C, NLOC], dt.float32)
            for p in range(NCHUNK):
                nc.sync.dma_start(
                    out=x_sb[:, p * 512:(p + 1) * 512],
                    in_=x_d[:, p * 512:(p + 1) * 512],
                )

            # ---- conv + relu + bn_stats ----
            ypool = ctx.enter_context(tc.tile_pool(name="ypool", bufs=2))
            y_int = ypool.tile([C, NLOC], dt.float32, tag="big")
            y_ext = ypool.tile([C, NLOC], dt.float32, tag="big")
            stats = ctx.enter_context(tc.tile_pool(name="stats", bufs=1))
            st_i = stats.tile([C, NCHUNK, 6], dt.float32)
            st_e = stats.tile([C, NCHUNK, 6], dt.float32)

            with tc.tile_pool(name="convps", bufs=4, space="PSUM") as convps:
                for (wT, bcol, y_sb, st) in [(wTi, cols["bi"], y_int, st_i),
                                             (wTe, cols["be"], y_ext, st_e)]:
                    for p in range(NCHUNK):
                        sl = slice(p * 512, (p + 1) * 512)
                        ps = convps.tile([C, 512], dt.float32, tag="cps")
                        nc.tensor.matmul(ps[:, :], wT[:, :], x_sb[:, sl],
                                         start=True, stop=True)
                        nc.scalar.activation(y_sb[:, sl], ps[:, :], Act.Relu,
                                             bias=bcol[:, :], scale=1.0)
                        nc.vector.bn_stats(st[:, p, :], y_sb[:, sl])

            # S_x[c, bj] = sum_t x  (emitted here; scheduler overlaps)
            S_x = small.tile([C, NBJ], dt.float32)
            nc.vector.tensor_reduce(S_x[:, :], x_sb.rearrange("c (bj t) -> c bj t", t=T),
                                    axis=mybir.AxisListType.X, op=Alu.add)

            # ---- local sums + AllReduce #1 ----
            mv_i = small.tile([C, 2], dt.float32)
            mv_e = small.tile([C, 2], dt.float32)
            nc.vector.bn_aggr(mv_i[:, :], st_i[:, :, :])
            nc.vector.bn_aggr(mv_e[:, :], st_e[:, :, :])
            sums4 = small.tile([C, 4], dt.float32)
            tmpc = small.tile([C, 1], dt.float32)
            for k, mv in [(0, mv_i), (2, mv_e)]:
                nc.vector.tensor_scalar_mul(sums4[:, k:k + 1], mv[:, 0:1], float(NLOC))
                nc.vector.tensor_mul(tmpc[:, :], mv[:, 0:1], mv[:, 0:1])
                nc.vector.tensor_add(tmpc[:, :], tmpc[:, :], mv[:, 1:2])
                nc.vector.tensor_scalar_mul(sums4[:, k + 1:k + 2], tmpc[:, :], float(NLOC))

            ar1_in = dram.tile([C, 4], dt.float32)
            ar1_out = dram.tile([C, 4], dt.float32, addr_space="Shared")
            nc.sync.dma_start(out=ar1_in[:, :], in_=sums4[:, :])
            nc.gpsimd.collective_compute(
                "AllReduce", Alu.add,
                replica_groups=[list(range(NCORES))],
                ins=[ar1_in[:, :].opt()], outs=[ar1_out[:, :].opt()],
            )
            g1 = small.tile([C, 4], dt.float32)
            nc.sync.dma_start(out=g1[:, :], in_=ar1_out[:, :])

            # ---- global affine params per branch ----
            def bn_params(goff, gcol, btcol, tag):
                mean = small.tile([C, 1], dt.float32, name="mean_" + tag)
                ex2 = small.tile([C, 1], dt.float32, name="ex2_" + tag)
                var = small.tile([C, 1], dt.float32, name="var_" + tag)
                nc.vector.tensor_scalar_mul(mean[:, :], g1[:, goff:goff + 1], 1.0 / NGLOB)
                nc.vector.tensor_scalar_mul(ex2[:, :], g1[:, goff + 1:goff + 2], 1.0 / NGLOB)
                nc.vector.tensor_mul(var[:, :], mean[:, :], mean[:, :])
                nc.vector.tensor_sub(var[:, :], ex2[:, :], var[:, :])
                # rsqrt(var+eps) with 2 Newton steps (ACT sqrt table is low precision)
                sd = small.tile([C, 1], dt.float32, name="sd_" + tag)
                veps = small.tile([C, 1], dt.float32, name="veps_" + tag)
                nc.vector.tensor_scalar_add(veps[:, :], var[:, :], EPS)
                nc.scalar.activation(sd[:, :], veps[:, :], Act.Sqrt)
                r = small.tile([C, 1], dt.float32, name="r_" + tag)
                nc.vector.reciprocal(r[:, :], sd[:, :])
                t1 = small.tile([C, 1], dt.float32, name="t1_" + tag)
                for _ in range(2):
                    nc.vector.tensor_mul(t1[:, :], r[:, :], r[:, :])
                    nc.vector.tensor_mul(t1[:, :], t1[:, :], veps[:, :])
                    nc.vector.tensor_scalar(t1[:, :], t1[:, :], -0.5, 1.5,
                                            op0=Alu.mult, op1=Alu.add)
                    nc.vector.tensor_mul(r[:, :], r[:, :], t1[:, :])
                s_ = small.tile([C, 1], dt.float32, name="s_" + tag)
                t_ = small.tile([C, 1], dt.float32, name="t_" + tag)
                nc.vector.tensor_mul(s_[:, :], r[:, :], gcol[:, :])
                nc.vector.tensor_mul(t1[:, :], mean[:, :], s_[:, :])
                nc.vector.tensor_sub(t_[:, :], btcol[:, :], t1[:, :])
                return s_, t_

            s_i, t_i = bn_params(0, cols["gi"], cols["bti"], "i")
            s_e, t_e = bn_params(2, cols["ge"], cols["bte"], "e")
            u_i = small.tile([C, 1], dt.float32)
            u_e = small.tile([C, 1], dt.float32)
            nc.vector.tensor_mul(u_i[:, :], s_i[:, :], cols["wfi"][:, :])
            nc.vector.tensor_mul(u_e[:, :], s_e[:, :], cols["wfe"][:, :])

            # k0 = sum_c(wfi*t_i) + sum_c(wfe*t_e) + b_fus, broadcast to all partitions
            zc = small.tile([C, 1], dt.float32)
            z2 = small.tile([C, 1], dt.float32)
            nc.vector.tensor_mul(zc[:, :], t_i[:, :], cols["wfi"][:, :])
            nc.vector.tensor_mul(z2[:, :], t_e[:, :], cols["wfe"][:, :])
            nc.vector.tensor_add(zc[:, :], zc[:, :], z2[:, :])
            k0 = small.tile([C, 1], dt.float32)
            with tc.tile_pool(name="kps", bufs=1, space="PSUM") as kps:
                psk = kps.tile([C, 1], dt.float32)
                nc.tensor.matmul(psk[:, :], ones_f[:, :], zc[:, :], start=True, stop=True)
                nc.vector.tensor_add(k0[:, :], psk[:, :], cols["bfus"][:, :])

            # ---- a-dots: a_rows[p, h*256+s] for bj=2p+h ----
            # psum rows land on partition 0; engines cannot write partition p!=0/32/64/96,
            # so each row bounces through DRAM and the (25,512) layout comes back by DMA.
            a_i = small.tile([J, 512], dt.float32)
            a_e = small.tile([J, 512], dt.float32)
            a0d_i = dram.tile([J, 512], dt.float32)
            a0d_e = dram.tile([J, 512], dt.float32)
            arow = ctx.enter_context(tc.tile_pool(name="arow", bufs=2))
            with tc.tile_pool(name="aps", bufs=4, space="PSUM") as aps:
                for p in range(NCHUNK):
                    sl = slice(p * 512, (p + 1) * 512)
                    psa = aps.tile([1, 512], dt.float32, tag="aps")
                    nc.tensor.matmul(psa[:, :], u_i[:, :], y_int[:, sl],
                                     start=True, stop=True)
                    ta = arow.tile([1, 512], dt.float32, tag="ar")
                    nc.scalar.copy(ta[:, :], psa[:, :])
                    nc.sync.dma_start(out=a0d_i[p:p + 1, :], in_=ta[:, :])
                    psb = aps.tile([1, 512], dt.float32, tag="aps")
                    nc.tensor.matmul(psb[:, :], u_e[:, :], y_ext[:, sl],
                                     start=True, stop=True)
                    tb = arow.tile([1, 512], dt.float32, tag="ar")
                    nc.vector.tensor_copy(tb[:, :], psb[:, :])
                    nc.sync.dma_start(out=a0d_e[p:p + 1, :], in_=tb[:, :])
            nc.sync.dma_start(out=a_i[:, :], in_=a0d_i[:, :])
            nc.sync.dma_start(out=a_e[:, :], in_=a0d_e[:, :])

            # ---- top-3 + sorted index triples (per half h: bj = 2p+h) ----
            idxp = ctx.enter_context(tc.tile_pool(name="idxp", bufs=1))
            iota = idxp.tile([J, T], dt.int32)
            nc.gpsimd.iota(iota[:, :], pattern=[[1, T]], base=0, channel_multiplier=0)
            iota_f = idxp.tile([J, T], dt.float32)
            nc.vector.tensor_copy(iota_f[:, :], iota[:, :])
            jo = idxp.tile([J, 1], dt.int32)
            nc.gpsimd.iota(jo[:, :], pattern=[[0, 1]], base=0, channel_multiplier=512)
            jof = idxp.tile([J, 1], dt.float32)
            nc.vector.tensor_copy(jof[:, :], jo[:, :])
            joff = []
            msk = idxp.tile([J, 1], dt.float32)
            for h in range(2):
                jt = idxp.tile([J, 1], dt.float32, name=f"joff{h}")
                nc.vector.tensor_scalar_add(jt[:, :], jof[:, :], float(256 * h))
                # rows with bj = 2p+h >= 25 wrap to sample b=1: subtract 6400
                lo = 13 - h
                nc.vector.tensor_scalar(msk[:, :], jof[:, :], float(512 * lo), None,
                                        op0=Alu.is_ge)
                nc.vector.scalar_tensor_tensor(jt[:, :], msk[:, :], -6400.0, jt[:, :],
                                               op0=Alu.mult, op1=Alu.add)
                joff.append(jt)

            idx_i32 = idxp.tile([J, 2 * T * 3], dt.int32)     # (25, 1536)
            gidx16 = idxp.tile([J, 2 * T * 3], dt.int16)
            mx8 = idxp.tile([J, 8], dt.float32)
            ix8 = idxp.tile([J, 8], dt.uint32)
            qb = idxp.tile([J, T], dt.float32)
            em = idxp.tile([J, T], dt.int32)
            e2 = idxp.tile([J, T], dt.int32)
            planes = ctx.enter_context(tc.tile_pool(name="planes", bufs=3))
            ccols = ctx.enter_context(tc.tile_pool(name="ccols", bufs=8))

            for h in range(2):
                half = slice(h * T, (h + 1) * T)
                nc.vector.max(mx8[:, :], a_i[:, half])
                nc.vector.max_index(ix8[:, :], mx8[:, :], a_i[:, half])
                i1 = ccols.tile([J, 1], dt.float32, tag="cc")
                i2 = ccols.tile([J, 1], dt.float32, tag="cc")
                i3 = ccols.tile([J, 1], dt.float32, tag="cc")
                nc.vector.tensor_copy(i1[:, :], ix8[:, 0:1])
                nc.vector.tensor_copy(i2[:, :], ix8[:, 1:2])
                nc.vector.tensor_copy(i3[:, :], ix8[:, 2:3])
                m1 = ccols.tile([J, 1], dt.float32, tag="cc")
                m2 = ccols.tile([J, 1], dt.float32, tag="cc")
                nc.vector.tensor_tensor(m1[:, :], i1[:, :], i2[:, :], op=Alu.min)
                nc.vector.tensor_tensor(m2[:, :], i1[:, :], i2[:, :], op=Alu.max)
                q0 = ccols.tile([J, 1], dt.float32, tag="cc")
                q2 = ccols.tile([J, 1], dt.float32, tag="cc")
                q1 = ccols.tile([J, 1], dt.float32, tag="cc")
                nc.vector.tensor_tensor(q0[:, :], m1[:, :], i3[:, :], op=Alu.min)
                nc.vector.tensor_tensor(q2[:, :], m2[:, :], i3[:, :], op=Alu.max)
                nc.vector.tensor_add(q1[:, :], i1[:, :], i2[:, :])
                nc.vector.tensor_add(q1[:, :], q1[:, :], i3[:, :])
                nc.vector.tensor_sub(q1[:, :], q1[:, :], q0[:, :])
                nc.vector.tensor_sub(q1[:, :], q1[:, :], q2[:, :])

                # special-row mask: s==i1 or s==i2
                nc.vector.tensor_scalar(em[:, :], iota_f[:, :], i1[:, :], None, op0=Alu.is_equal)
                nc.vector.tensor_scalar(e2[:, :], iota_f[:, :], i2[:, :], None, op0=Alu.is_equal)
                nc.vector.tensor_add(em[:, :], em[:, :], e2[:, :])

                for l, qq in enumerate([q0, q1, q2]):
                    pl = planes.tile([J, T], dt.float32, tag="pl")
                    if l == 0:
                        nc.vector.tensor_scalar(pl[:, :], iota_f[:, :], m1[:, :], None, op0=Alu.min)
                    elif l == 2:
                        nc.vector.tensor_scalar(pl[:, :], iota_f[:, :], m2[:, :], None, op0=Alu.max)
                    else:
                        nc.vector.tensor_scalar(pl[:, :], iota_f[:, :], m1[:, :], m2[:, :],
                                                op0=Alu.max, op1=Alu.min)
                    # qb = broadcast(qq); overwrite special rows
                    nc.vector.tensor_scalar(qb[:, :], iota_f[:, :], 0.0, qq[:, :],
                                            op0=Alu.mult, op1=Alu.add)
                    nc.vector.copy_predicated(pl[:, :], em[:, :], qb[:, :])
                    # interleave into (s,l) layout: idx3 output (int32 cast)
                    dst = bass.AP(tensor=idx_i32.tensor, offset=idx_i32.offset + h * T * 3 + l,
                                  ap=[list(idx_i32.ap[0]), [3, T]])
                    nc.vector.tensor_copy(dst, pl[:, :])
                    # gather index with +j*256 offset (int16 cast)
                    gdst = bass.AP(tensor=gidx16.tensor, offset=gidx16.offset + h * T * 3 + l,
                                   ap=[list(gidx16.ap[0]), [3, T]])
                    nc.vector.tensor_scalar_add(gdst, pl[:, :], joff[h][:, :])

            # idx3 output DMA: rows are bj pairs -> flat (b j s l)
            idx3_flat = bass.AP(tensor=idx3_d, offset=0, ap=[[1536, J], [1, 1536]])
            nc.sync.dma_start(out=idx3_flat, in_=idx_i32[:, :])

            # gather-index scratch roundtrip: wrap 16-partition layout for ap_gather
            scratch = dram.tile([J, 1536], dt.int16)
            nc.sync.dma_start(out=scratch[:, :], in_=gidx16[:, :])
            widx = idxp.tile([C, 2, 1200], dt.int16)
            scratch2 = dram.tile([BL, C, 1200], dt.int16)
            for b_ in range(BL):
                # wrap-transpose (1200,16) -> (16,1200) for ap_gather's per-core layout
                src = bass.AP(tensor=scratch.tensor,
                              offset=scratch.offset + b_ * 19200,
                              ap=[[1, 16], [16, 1200]])
                nc.sync.dma_start(out=scratch2[b_, 0:16, :], in_=src)
                for k in range(1, 8):
                    nc.sync.dma_start(out=scratch2[b_, 16 * k:16 * (k + 1), :],
                                      in_=scratch2[b_, 0:16, :])
                nc.sync.dma_start(out=widx[:, b_, :], in_=scratch2[b_, :, :])

            # ---- gather path values + write out ----
            gpool = ctx.enter_context(tc.tile_pool(name="gpool", bufs=2))
            path_flat = path_d.rearrange("b c j t l -> b c (j t l)")
            for b_ in range(BL):
                for jc in range(0, J, 2):
                    nj = min(2, J - jc)
                    nidx = nj * 768
                    gout = gpool.tile([C, 1536], dt.float32, tag="gout")
                    nc.gpsimd.ap_gather(
                        gout[:, :nidx],
                        x_sb[:, b_ * J * T:(b_ + 1) * J * T],
                        widx[:, b_, jc * 48:jc * 48 + nidx // 16],
                        channels=C, num_elems=J * T, d=1, num_idxs=nidx,
                    )
                    nc.sync.dma_start(
                        out=path_flat[b_, :, jc * 768:jc * 768 + nidx],
                        in_=gout[:, :nidx],
                    )

            # ---- D[c,bj] = sum_t a_int[t] x[c,t] ; bias_all = k0*S_x + D ----
            Dall = small.tile([C, NBJ], dt.float32)
            bias_all = small.tile([C, NBJ], dt.float32)
            djp = ctx.enter_context(tc.tile_pool(name="djp", bufs=2))
            rpool = ctx.enter_context(tc.tile_pool(name="rpool", bufs=3))
            with tc.tile_pool(name="bps1", bufs=2, space="PSUM") as bps1:
                for p in range(NCHUNK):
                    rhs = rpool.tile([1, 512], dt.float32, tag="rhs")
                    nc.sync.dma_start(out=rhs[:, :], in_=a0d_i[p:p + 1, :])
                    psd = bps1.tile([C, 512], dt.float32, tag="bps1")
                    nc.tensor.matmul(psd[:, :], ones_r[:, :].bitcast(dt.float32r),
                                     rhs[:, :].bitcast(dt.float32r),
                                     start=True, stop=True)
                    for h in range(2):
                        bj = 2 * p + h
                        dj = djp.tile([C, T], dt.float32, tag="dj")
                        nc.vector.scalar_tensor_tensor(
                            dj[:, :], x_sb[:, bj * T:(bj + 1) * T], 1.0,
                            psd[:, h * T:(h + 1) * T],
                            op0=Alu.mult, op1=Alu.mult,
                            accum_out=Dall[:, bj:bj + 1])
            nc.vector.scalar_tensor_tensor(bias_all[:, :], S_x[:, :], k0[:, :],
                                           Dall[:, :], op0=Alu.mult, op1=Alu.add)

            # ---- out_pre -> relu -> bn_stats ----
            out_sb = ypool.tile([C, NLOC], dt.float32, tag="big")
            ost = stats.tile([C, NBJ, 6], dt.float32)
            with tc.tile_pool(name="bps2", bufs=2, space="PSUM") as bps2:
                for p in range(NCHUNK):
                    rhs = rpool.tile([1, 512], dt.float32, tag="rhs")
                    nc.sync.dma_start(out=rhs[:, :], in_=a0d_e[p:p + 1, :])
                    pse = bps2.tile([C, 512], dt.float32, tag="bps2")
                    nc.tensor.matmul(pse[:, :], ones_r[:, :].bitcast(dt.float32r),
                                     rhs[:, :].bitcast(dt.float32r),
                                     start=True, stop=True)
                    for h in range(2):
                        bj = 2 * p + h
                        sl = slice(bj * T, (bj + 1) * T)
                        nc.scalar.activation(out_sb[:, sl], pse[:, h * T:(h + 1) * T],
                                             Act.Relu,
                                             bias=bias_all[:, bj:bj + 1],
                                             scale=S_x[:, bj:bj + 1])
                        nc.vector.bn_stats(ost[:, bj, :], out_sb[:, sl])

            # ---- AllReduce #2 + out affine + store ----
            mv_o = small.tile([C, 2], dt.float32)
            nc.vector.bn_aggr(mv_o[:, :], ost[:, :, :])
            sums2 = small.tile([C, 2], dt.float32)
            nc.vector.tensor_scalar_mul(sums2[:, 0:1], mv_o[:, 0:1], float(NLOC))
            nc.vector.tensor_mul(tmpc[:, :], mv_o[:, 0:1], mv_o[:, 0:1])
            nc.vector.tensor_add(tmpc[:, :], tmpc[:, :], mv_o[:, 1:2])
            nc.vector.tensor_scalar_mul(sums2[:, 1:2], tmpc[:, :], float(NLOC))
            ar2_in = dram.tile([C, 2], dt.float32)
            ar2_out = dram.tile([C, 2], dt.float32, addr_space="Shared")
            nc.sync.dma_start(out=ar2_in[:, :], in_=sums2[:, :])
            nc.gpsimd.collective_compute(
                "AllReduce", Alu.add,
                replica_groups=[list(range(NCORES))],
                ins=[ar2_in[:, :].opt()], outs=[ar2_out[:, :].opt()],
            )
            g2 = small.tile([C, 2], dt.float32)
            nc.sync.dma_start(out=g2[:, :], in_=ar2_out[:, :])
            # reuse g1 slots trick not needed; compute s_o/t_o
            g2w = small.tile([C, 4], dt.float32)
            nc.vector.tensor_copy(g2w[:, 0:2], g2[:, :])
            g1_save = g1
            g1 = g2w
            s_o, t_o = bn_params(0, cols["go"], cols["bto"], "o")
            g1 = g1_save

            for p in range(NCHUNK):
                sl = slice(p * 512, (p + 1) * 512)
                nc.vector.tensor_scalar(out_sb[:, sl], out_sb[:, sl],
                                        s_o[:, :], t_o[:, :],
                                        op0=Alu.mult, op1=Alu.add)
            for bi_ in range(BL):
                for j in range(J):
                    bj = bi_ * J + j
                    nc.sync.dma_start(
                        out=out_d[bi_, :, j, :],
                        in_=out_sb[:, bj * T:(bj + 1) * T],
                    )

    # bacc lowering: wait-splitting onto ldweights, library loads for ap_gather,
    # extended-ISA codegen, nop fusion, register allocation
    nc.compile()
    return nc


_CACHE = {}


def _get_nc():
    if "nc" not in _CACHE:
        _CACHE["nc"] = build_bass()
    return _CACHE["nc"]


def _in_maps(inputs):
    f32 = lambda a: np.ascontiguousarray(np.asarray(a), dtype=np.float32)
    x = f32(inputs["x"])
    wTi = np.ascontiguousarray(f32(inputs["w_int"]).T)
    wTe = np.ascontiguousarray(f32(inputs["w_ext"]).T)
    wf = f32(inputs["w_fus"])
    bfus_col = np.full((C,), float(np.asarray(inputs["b_fus"])), dtype=np.float32)
    common = dict(
        wTi=wTi, wTe=wTe,
        bi=f32(inputs["b_int"]), be=f32(inputs["b_ext"]),
        wfi=np.ascontiguousarray(wf[:C]), wfe=np.ascontiguousarray(wf[C:]),
        bfus=bfus_col,
        gi=f32(inputs["g_int"]), bti=f32(inputs["beta_int"]),
        ge=f32(inputs["g_ext"]), bte=f32(inputs["beta_ext"]),
        go=f32(inputs["g_out"]), bto=f32(inputs["beta_out"]),
    )
    maps = []
    for d in range(NCORES):
        m = dict(common)
        m["x"] = np.ascontiguousarray(
            x[d * BL:(d + 1) * BL].transpose(1, 0, 2, 3).reshape(C, NLOC))
        maps.append(m)
    return maps


def _install_ntff_shim():
    """Register the axon NTFF profile hook (missing antenv.axon_hooks shim)."""
    import sys, types
    if "antenv.axon_hooks" in sys.modules:
        return
    try:
        sys.path.insert(0, "/root/.axon_site")
        from trn_agent_boot.trn_boot import _ntff_profile_via_ctypes
        hook = _ntff_profile_via_ctypes("/opt/axon/libaxon_pjrt.so")
        mod = types.ModuleType("antenv.axon_hooks")
        mod.get_axon_ntff_profile_hook = lambda: hook
        mod.set_axon_ntff_profile_hook = lambda h: None
        sys.modules["antenv.axon_hooks"] = mod
        import concourse.bass_utils as bu
        bu.upload_artifacts = lambda d: d  # no artifact bucket in this container
    except Exception as e:  # pragma: no cover
        print("ntff shim install failed:", e)


def run_spmd(inputs, trace=False):
    if trace:
        _install_ntff_shim()
    nc = _get_nc()
    res = run_bass_kernel_spmd(nc, _in_maps(inputs), list(range(NCORES)), trace=trace)
    return res


def kernel(**inputs):
    res = run_spmd(inputs, trace=False)
    outs = res.results
    out = np.concatenate([outs[d]["out"] for d in range(NCORES)], axis=0)
    path = np.concatenate([outs[d]["path"] for d in range(NCORES)], axis=0)
    idx3 = np.concatenate([outs[d]["idx3"] for d in range(NCORES)], axis=0)
    idx3 = idx3.astype(np.int32, copy=False)
    idx_full = np.broadcast_to(idx3[:, None], (B, C, J, T, 3))
    return out, path, idx_full


# revision 30
# speedup vs baseline: 1.3067x; 1.3067x over previous
"""Trainium2 Bass kernel for nn_ConvOffset2D_nonlocal2 (topk_masking).

Structure exploited:
  matrix[b,j,s,t] = a_ext[b,j,s] + a_int[b,j,t] + b_fus   (rank-1 + broadcast)
so
  out_pre[b,c,j,s] = (a_ext[s]+k)*S_x[c] + D[c],  S_x = sum_t x, D = sum_t a_int[t] x[:,t]
and the masked top-2 over t of matrix[...,s,:] is independent of s (row offset
does not change ranks): per (b,j) it is the top-3 of a_int with two special rows.

Data-parallel over B across 8 cores; sync-BN via two tiny in-kernel AllReduces.
"""

import numpy as np

import concourse.bass as bass
import concourse.mybir as mybir
import concourse.tile as tile
from concourse import bacc, library_config
from concourse.bass_utils import run_bass_kernel_spmd

dt = mybir.dt
Alu = mybir.AluOpType
Act = mybir.ActivationFunctionType

B, C, J, T = 16, 128, 25, 256
NCORES = 8
BL = B // NCORES            # 2 samples per core
NBJ = BL * J                # 50 (b,j) pairs per core
NLOC = BL * J * T           # 12800 free elements per core
NGLOB = float(B * J * T)    # 102400
EPS = 1e-5
NCHUNK = NLOC // 512        # 25 conv chunks of 512


def build_bass():
    nc = bacc.Bacc(None, target_bir_lowering=False)

    # ---- I/O ----
    x_d = nc.declare_dram_parameter("x", [C, NLOC], dt.float32, isOutput=False)
    wTi_d = nc.declare_dram_parameter("wTi", [C, C], dt.float32, isOutput=False)
    wTe_d = nc.declare_dram_parameter("wTe", [C, C], dt.float32, isOutput=False)
    bi_d = nc.declare_dram_parameter("bi", [C], dt.float32, isOutput=False)
    be_d = nc.declare_dram_parameter("be", [C], dt.float32, isOutput=False)
    wfi_d = nc.declare_dram_parameter("wfi", [C], dt.float32, isOutput=False)
    wfe_d = nc.declare_dram_parameter("wfe", [C], dt.float32, isOutput=False)
    bfus_d = nc.declare_dram_parameter("bfus", [C], dt.float32, isOutput=False)
    gi_d = nc.declare_dram_parameter("gi", [C], dt.float32, isOutput=False)
    bti_d = nc.declare_dram_parameter("bti", [C], dt.float32, isOutput=False)
    ge_d = nc.declare_dram_parameter("ge", [C], dt.float32, isOutput=False)
    bte_d = nc.declare_dram_parameter("bte", [C], dt.float32, isOutput=False)
    go_d = nc.declare_dram_parameter("go", [C], dt.float32, isOutput=False)
    bto_d = nc.declare_dram_parameter("bto", [C], dt.float32, isOutput=False)

    out_d = nc.declare_dram_parameter("out", [BL, C, J, T], dt.float32, isOutput=True)
    path_d = nc.declare_dram_parameter("path", [BL, C, J, T, 3], dt.float32, isOutput=True)
    idx3_d = nc.declare_dram_parameter("idx3", [BL, J, T, 3], dt.int32, isOutput=True)

    col = lambda d: d.rearrange("(c one) -> c one", one=1)

    with tile.TileContext(nc) as tc:
        from contextlib import ExitStack

        ctx = ExitStack()
        with ctx:
            consts = ctx.enter_context(tc.tile_pool(name="consts", bufs=1))
            dram = ctx.enter_context(tc.tile_pool(name="dram", bufs=1, space="DRAM"))
            small = ctx.enter_context(tc.tile_pool(name="small", bufs=1))

            # ---- constants to SBUF ----
            wTi = consts.tile([C, C], dt.float32)
            wTe = consts.tile([C, C], dt.float32)
            nc.sync.dma_start(out=wTi[:, :], in_=wTi_d[:, :])
            nc.sync.dma_start(out=wTe[:, :], in_=wTe_d[:, :])
            cols = {}
            for name, d in [("bi", bi_d), ("be", be_d), ("wfi", wfi_d), ("wfe", wfe_d),
                            ("bfus", bfus_d), ("gi", gi_d), ("bti", bti_d), ("ge", ge_d),
                            ("bte", bte_d), ("go", go_d), ("bto", bto_d)]:
                t_ = consts.tile([C, 1], dt.float32, name="c_" + name)
                nc.sync.dma_start(out=t_[:, :], in_=col(d))
                cols[name] = t_
            ones_r = consts.tile([1, C], dt.float32)
            nc.vector.memset(ones_r[:, :], 1.0)
            ones_f = consts.tile([C, C], dt.float32)
            nc.vector.memset(ones_f[:, :], 1.0)

            # ---- x load: (c, b j t) ----
            xpool = ctx.enter_context(tc.tile_pool(name="xpool", bufs=1))
            x_sb = xpool.tile([C, NLOC], dt.float32)
            for p in range(NCHUNK):
                nc.sync.dma_start(
                    out=x_sb[:, p * 512:(p + 1) * 512],
                    in_=x_d[:, p * 512:(p + 1) * 512],
                )

            # ---- conv + relu + bn_stats ----
            ypool = ctx.enter_context(tc.tile_pool(name="ypool", bufs=2))
            y_int = ypool.tile([C, NLOC], dt.float32, tag="big")
            y_ext = ypool.tile([C, NLOC], dt.float32, tag="big")
            stats = ctx.enter_context(tc.tile_pool(name="stats", bufs=1))
            st_i = stats.tile([C, NCHUNK, 6], dt.float32)
            st_e = stats.tile([C, NCHUNK, 6], dt.float32)

            with tc.tile_pool(name="convps", bufs=4, space="PSUM") as convps:
                for (wT, bcol, y_sb, st) in [(wTi, cols["bi"], y_int, st_i),
                                             (wTe, cols["be"], y_ext, st_e)]:
                    for p in range(NCHUNK):
                        sl = slice(p * 512, (p + 1) * 512)
                        ps = convps.tile([C, 512], dt.float32, tag="cps")
                        nc.tensor.matmul(ps[:, :], wT[:, :], x_sb[:, sl],
                                         start=True, stop=True)
                        nc.scalar.activation(y_sb[:, sl], ps[:, :], Act.Relu,
                                             bias=bcol[:, :], scale=1.0)
                        nc.vector.bn_stats(st[:, p, :], y_sb[:, sl])

            # S_x[c, bj] = sum_t x  (emitted here; scheduler overlaps)
            S_x = small.tile([C, NBJ], dt.float32)
            nc.vector.tensor_reduce(S_x[:, :], x_sb.rearrange("c (bj t) -> c bj t", t=T),
                                    axis=mybir.AxisListType.X, op=Alu.add)

            # ---- local sums + AllReduce #1 ----
            mv_i = small.tile([C, 2], dt.float32)
            mv_e = small.tile([C, 2], dt.float32)
            nc.vector.bn_aggr(mv_i[:, :], st_i[:, :, :])
            nc.vector.bn_aggr(mv_e[:, :], st_e[:, :, :])
            sums4 = small.tile([C, 4], dt.float32)
            tmpc = small.tile([C, 1], dt.float32)
            for k, mv in [(0, mv_i), (2, mv_e)]:
                nc.vector.tensor_scalar_mul(sums4[:, k:k + 1], mv[:, 0:1], float(NLOC))
                nc.vector.tensor_mul(tmpc[:, :], mv[:, 0:1], mv[:, 0:1])
                nc.vector.tensor_add(tmpc[:, :], tmpc[:, :], mv[:, 1:2])
                nc.vector.tensor_scalar_mul(sums4[:, k + 1:k + 2], tmpc[:, :], float(NLOC))

            ar1_in = dram.tile([C, 4], dt.float32)
            ar1_out = dram.tile([C, 4], dt.float32, addr_space="Shared")
            nc.sync.dma_start(out=ar1_in[:, :], in_=sums4[:, :])
            nc.gpsimd.collective_compute(
                "AllReduce", Alu.add,
                replica_groups=[list(range(NCORES))],
                ins=[ar1_in[:, :].opt()], outs=[ar1_out[:, :].opt()],
            )
            g1 = small.tile([C, 4], dt.float32)
            nc.sync.dma_start(out=g1[:, :], in_=ar1_out[:, :])

            # ---- global affine params per branch ----
            def bn_params(goff, gcol, btcol, tag):
                mean = small.tile([C, 1], dt.float32, name="mean_" + tag)
                ex2 = small.tile([C, 1], dt.float32, name="ex2_" + tag)
                var = small.tile([C, 1], dt.float32, name="var_" + tag)
                nc.vector.tensor_scalar_mul(mean[:, :], g1[:, goff:goff + 1], 1.0 / NGLOB)
                nc.vector.tensor_scalar_mul(ex2[:, :], g1[:, goff + 1:goff + 2], 1.0 / NGLOB)
                nc.vector.tensor_mul(var[:, :], mean[:, :], mean[:, :])
                nc.vector.tensor_sub(var[:, :], ex2[:, :], var[:, :])
                # rsqrt(var+eps) with 2 Newton steps (ACT sqrt table is low precision)
                sd = small.tile([C, 1], dt.float32, name="sd_" + tag)
                veps = small.tile([C, 1], dt.float32, name="veps_" + tag)
                nc.vector.tensor_scalar_add(veps[:, :], var[:, :], EPS)
                nc.scalar.activation(sd[:, :], veps[:, :], Act.Sqrt)
                r = small.tile([C, 1], dt.float32, name="r_" + tag)
                nc.vector.reciprocal(r[:, :], sd[:, :])
                t1 = small.tile([C, 1], dt.float32, name="t1_" + tag)
                for _ in range(2):
                    nc.vector.tensor_mul(t1[:, :], r[:, :], r[:, :])
                    nc.vector.tensor_mul(t1[:, :], t1[:, :], veps[:, :])
                    nc.vector.tensor_scalar(t1[:, :], t1[:, :], -0.5, 1.5,
                                            op0=Alu.mult, op1=Alu.add)
                    nc.vector.tensor_mul(r[:, :], r[:, :], t1[:, :])
                s_ = small.tile([C, 1], dt.float32, name="s_" + tag)
                t_ = small.tile([C, 1], dt.float32, name="t_" + tag)
                nc.vector.tensor_mul(s_[:, :], r[:, :], gcol[:, :])
                nc.vector.tensor_mul(t1[:, :], mean[:, :], s_[:, :])
                nc.vector.tensor_sub(t_[:, :], btcol[:, :], t1[:, :])
                return s_, t_

            s_i, t_i = bn_params(0, cols["gi"], cols["bti"], "i")
            s_e, t_e = bn_params(2, cols["ge"], cols["bte"], "e")
            u_i = small.tile([C, 1], dt.float32)
            u_e = small.tile([C, 1], dt.float32)
            nc.vector.tensor_mul(u_i[:, :], s_i[:, :], cols["wfi"][:, :])
            nc.vector.tensor_mul(u_e[:, :], s_e[:, :], cols["wfe"][:, :])

            # k0 = sum_c(wfi*t_i) + sum_c(wfe*t_e) + b_fus, broadcast to all partitions
            zc = small.tile([C, 1], dt.float32)
            z2 = small.tile([C, 1], dt.float32)
            nc.vector.tensor_mul(zc[:, :], t_i[:, :], cols["wfi"][:, :])
            nc.vector.tensor_mul(z2[:, :], t_e[:, :], cols["wfe"][:, :])
            nc.vector.tensor_add(zc[:, :], zc[:, :], z2[:, :])
            k0 = small.tile([C, 1], dt.float32)
            with tc.tile_pool(name="kps", bufs=1, space="PSUM") as kps:
                psk = kps.tile([C, 1], dt.float32)
                nc.tensor.matmul(psk[:, :], ones_f[:, :], zc[:, :], start=True, stop=True)
                nc.vector.tensor_add(k0[:, :], psk[:, :], cols["bfus"][:, :])

            # ---- a-dots: a_rows[p, h*256+s] for bj=2p+h ----
            # psum rows land on partition 0; engines cannot write partition p!=0/32/64/96,
            # so each row bounces through DRAM and the (25,512) layout comes back by DMA.
            a_i = small.tile([J, 512], dt.float32)
            a_e = small.tile([J, 512], dt.float32)
            a0d_i = dram.tile([J, 512], dt.float32)
            a0d_e = dram.tile([J, 512], dt.float32)
            arow = ctx.enter_context(tc.tile_pool(name="arow", bufs=2))
            with tc.tile_pool(name="aps", bufs=4, space="PSUM") as aps:
                for p in range(NCHUNK):
                    sl = slice(p * 512, (p + 1) * 512)
                    psa = aps.tile([1, 512], dt.float32, tag="aps")
                    nc.tensor.matmul(psa[:, :], u_i[:, :], y_int[:, sl],
                                     start=True, stop=True)
                    ta = arow.tile([1, 512], dt.float32, tag="ar")
                    nc.scalar.copy(ta[:, :], psa[:, :])
                    nc.sync.dma_start(out=a0d_i[p:p + 1, :], in_=ta[:, :])
                    psb = aps.tile([1, 512], dt.float32, tag="aps")
                    nc.tensor.matmul(psb[:, :], u_e[:, :], y_ext[:, sl],
                                     start=True, stop=True)
                    tb = arow.tile([1, 512], dt.float32, tag="ar")
                    nc.vector.tensor_copy(tb[:, :], psb[:, :])
                    nc.sync.dma_start(out=a0d_e[p:p + 1, :], in_=tb[:, :])
            nc.sync.dma_start(out=a_i[:, :], in_=a0d_i[:, :])
            nc.sync.dma_start(out=a_e[:, :], in_=a0d_e[:, :])

            # ---- top-3 + sorted index triples (per half h: bj = 2p+h) ----
            idxp = ctx.enter_context(tc.tile_pool(name="idxp", bufs=1))
            iota = idxp.tile([J, T], dt.int32)
            nc.gpsimd.iota(iota[:, :], pattern=[[1, T]], base=0, channel_multiplier=0)
            iota_f = idxp.tile([J, T], dt.float32)
            nc.vector.tensor_copy(iota_f[:, :], iota[:, :])
            jo = idxp.tile([J, 1], dt.int32)
            nc.gpsimd.iota(jo[:, :], pattern=[[0, 1]], base=0, channel_multiplier=512)
            jof = idxp.tile([J, 1], dt.float32)
            nc.vector.tensor_copy(jof[:, :], jo[:, :])
            joff = []
            msk = idxp.tile([J, 1], dt.float32)
            for h in range(2):
                jt = idxp.tile([J, 1], dt.float32, name=f"joff{h}")
                nc.vector.tensor_scalar_add(jt[:, :], jof[:, :], float(256 * h))
                # rows with bj = 2p+h >= 25 wrap to sample b=1: subtract 6400
                lo = 13 - h
                nc.vector.tensor_scalar(msk[:, :], jof[:, :], float(512 * lo), None,
                                        op0=Alu.is_ge)
                nc.vector.scalar_tensor_tensor(jt[:, :], msk[:, :], -6400.0, jt[:, :],
                                               op0=Alu.mult, op1=Alu.add)
                joff.append(jt)

            idx_i32 = idxp.tile([J, 2 * T * 3], dt.int32)     # (25, 1536)
            gidx16 = idxp.tile([J, 2 * T * 3], dt.int16)
            mx8 = idxp.tile([J, 8], dt.float32)
            ix8 = idxp.tile([J, 8], dt.uint32)
            qb = idxp.tile([J, T], dt.float32)
            em = idxp.tile([J, T], dt.int32)
            e2 = idxp.tile([J, T], dt.int32)
            planes = ctx.enter_context(tc.tile_pool(name="planes", bufs=3))
            ccols = ctx.enter_context(tc.tile_pool(name="ccols", bufs=8))

            for h in range(2):
                half = slice(h * T, (h + 1) * T)
                nc.vector.max(mx8[:, :], a_i[:, half])
                nc.vector.max_index(ix8[:, :], mx8[:, :], a_i[:, half])
                i1 = ccols.tile([J, 1], dt.float32, tag="cc")
                i2 = ccols.tile([J, 1], dt.float32, tag="cc")
                i3 = ccols.tile([J, 1], dt.float32, tag="cc")
                nc.vector.tensor_copy(i1[:, :], ix8[:, 0:1])
                nc.vector.tensor_copy(i2[:, :], ix8[:, 1:2])
                nc.vector.tensor_copy(i3[:, :], ix8[:, 2:3])
                m1 = ccols.tile([J, 1], dt.float32, tag="cc")
                m2 = ccols.tile([J, 1], dt.float32, tag="cc")
                nc.vector.tensor_tensor(m1[:, :], i1[:, :], i2[:, :], op=Alu.min)
                nc.vector.tensor_tensor(m2[:, :], i1[:, :], i2[:, :], op=Alu.max)
                q0 = ccols.tile([J, 1], dt.float32, tag="cc")
                q2 = ccols.tile([J, 1], dt.float32, tag="cc")
                q1 = ccols.tile([J, 1], dt.float32, tag="cc")
                nc.vector.tensor_tensor(q0[:, :], m1[:, :], i3[:, :], op=Alu.min)
                nc.vector.tensor_tensor(q2[:, :], m2[:, :], i3[:, :], op=Alu.max)
                nc.vector.tensor_add(q1[:, :], i1[:, :], i2[:, :])
                nc.vector.tensor_add(q1[:, :], q1[:, :], i3[:, :])
                nc.vector.tensor_sub(q1[:, :], q1[:, :], q0[:, :])
                nc.vector.tensor_sub(q1[:, :], q1[:, :], q2[:, :])

                # special-row mask: s==i1 or s==i2
                nc.vector.tensor_scalar(em[:, :], iota_f[:, :], i1[:, :], None, op0=Alu.is_equal)
                nc.vector.tensor_scalar(e2[:, :], iota_f[:, :], i2[:, :], None, op0=Alu.is_equal)
                nc.vector.tensor_add(em[:, :], em[:, :], e2[:, :])

                for l, qq in enumerate([q0, q1, q2]):
                    pl = planes.tile([J, T], dt.float32, tag="pl")
                    if l == 0:
                        nc.vector.tensor_scalar(pl[:, :], iota_f[:, :], m1[:, :], None, op0=Alu.min)
                    elif l == 2:
                        nc.vector.tensor_scalar(pl[:, :], iota_f[:, :], m2[:, :], None, op0=Alu.max)
                    else:
                        nc.vector.tensor_scalar(pl[:, :], iota_f[:, :], m1[:, :], m2[:, :],
                                                op0=Alu.max, op1=Alu.min)
                    # qb = broadcast(qq); overwrite special rows
                    nc.vector.tensor_scalar(qb[:, :], iota_f[:, :], 0.0, qq[:, :],
                                            op0=Alu.mult, op1=Alu.add)
                    nc.vector.copy_predicated(pl[:, :], em[:, :], qb[:, :])
                    # interleave into (s,l) layout: idx3 output (int32 cast)
                    dst = bass.AP(tensor=idx_i32.tensor, offset=idx_i32.offset + h * T * 3 + l,
                                  ap=[list(idx_i32.ap[0]), [3, T]])
                    nc.vector.tensor_copy(dst, pl[:, :])
                    # gather index with +j*256 offset (int16 cast)
                    gdst = bass.AP(tensor=gidx16.tensor, offset=gidx16.offset + h * T * 3 + l,
                                   ap=[list(gidx16.ap[0]), [3, T]])
                    nc.vector.tensor_scalar_add(gdst, pl[:, :], joff[h][:, :])

            # idx3 output DMA: rows are bj pairs -> flat (b j s l)
            idx3_flat = bass.AP(tensor=idx3_d, offset=0, ap=[[1536, J], [1, 1536]])
            nc.sync.dma_start(out=idx3_flat, in_=idx_i32[:, :])

            # gather-index wrap for ap_gather's per-core layout:
            # glist (flat, per b) -> widx[p, w] = glist[w*16 + p%16].
            # Done via the xbar transpose engine: read scratch as (1200, 128)
            # rows overlapping at stride 16; transposed rows 0:16 are the wrap.
            # Then replicate rows 0:16 to all 8 16-partition blocks via DRAM.
            scratch = dram.tile([J + 1, 1536], dt.int16)  # +1 row pad for overlap reads
            nc.sync.dma_start(out=scratch[0:J, :], in_=gidx16[:, :])
            widx = idxp.tile([C, 2, 1200], dt.int16)
            scratch2 = dram.tile([BL, 32, 1200], dt.int16)
            for b_ in range(BL):
                src = bass.AP(tensor=scratch.tensor,
                              offset=scratch.offset + b_ * 19200,
                              ap=[[16, 1200], [1, 128]])
                nc.sync.dma_start_transpose(out=widx[:, b_, :], in_=src)
                nc.sync.dma_start(out=scratch2[b_, 0:16, :], in_=widx[0:16, b_, :])
                # duplicate to 32 rows in DRAM, then fill all four 32-row groups
                nc.sync.dma_start(out=scratch2[b_, 16:32, :], in_=scratch2[b_, 0:16, :])
                for m in range(4):
                    nc.sync.dma_start(out=widx[32 * m:32 * (m + 1), b_, :],
                                      in_=scratch2[b_, :, :])

            # ---- gather path values + write out (SWDGE for the big writes) ----
            gpool = ctx.enter_context(tc.tile_pool(name="gpool", bufs=2))
            path_flat = path_d.rearrange("b c j t l -> b c (j t l)")
            for b_ in range(BL):
                for jc in range(0, J, 2):
                    nj = min(2, J - jc)
                    nidx = nj * 768
                    gout = gpool.tile([C, 1536], dt.float32, tag="gout")
                    nc.gpsimd.ap_gather(
                        gout[:, :nidx],
                        x_sb[:, b_ * J * T:(b_ + 1) * J * T],
                        widx[:, b_, jc * 48:jc * 48 + nidx // 16],
                        channels=C, num_elems=J * T, d=1, num_idxs=nidx,
                    )
                    nc.gpsimd.dma_start(
                        out=path_flat[b_, :, jc * 768:jc * 768 + nidx],
                        in_=gout[:, :nidx],
                    )

            # ---- D[c,bj] = sum_t a_int[t] x[c,t] ; bias_all = k0*S_x + D ----
            Dall = small.tile([C, NBJ], dt.float32)
            bias_all = small.tile([C, NBJ], dt.float32)
            djp = ctx.enter_context(tc.tile_pool(name="djp", bufs=2))
            rpool = ctx.enter_context(tc.tile_pool(name="rpool", bufs=3))
            with tc.tile_pool(name="bps1", bufs=2, space="PSUM") as bps1:
                for p in range(NCHUNK):
                    rhs = rpool.tile([1, 512], dt.float32, tag="rhs")
                    nc.sync.dma_start(out=rhs[:, :], in_=a0d_i[p:p + 1, :])
                    psd = bps1.tile([C, 512], dt.float32, tag="bps1")
                    nc.tensor.matmul(psd[:, :], ones_r[:, :].bitcast(dt.float32r),
                                     rhs[:, :].bitcast(dt.float32r),
                                     start=True, stop=True)
                    for h in range(2):
                        bj = 2 * p + h
                        dj = djp.tile([C, T], dt.float32, tag="dj")
                        nc.vector.scalar_tensor_tensor(
                            dj[:, :], x_sb[:, bj * T:(bj + 1) * T], 1.0,
                            psd[:, h * T:(h + 1) * T],
                            op0=Alu.mult, op1=Alu.mult,
                            accum_out=Dall[:, bj:bj + 1])
            nc.vector.scalar_tensor_tensor(bias_all[:, :], S_x[:, :], k0[:, :],
                                           Dall[:, :], op0=Alu.mult, op1=Alu.add)

            # ---- out_pre -> relu -> bn_stats ----
            out_sb = ypool.tile([C, NLOC], dt.float32, tag="big")
            ost = stats.tile([C, NBJ, 6], dt.float32)
            with tc.tile_pool(name="bps2", bufs=2, space="PSUM") as bps2:
                for p in range(NCHUNK):
                    rhs = rpool.tile([1, 512], dt.float32, tag="rhs")
                    nc.sync.dma_start(out=rhs[:, :], in_=a0d_e[p:p + 1, :])
                    pse = bps2.tile([C, 512], dt.float32, tag="bps2")
                    nc.tensor.matmul(pse[:, :], ones_r[:, :].bitcast(dt.float32r),
                                     rhs[:, :].bitcast(dt.float32r),
                                     start=True, stop=True)
                    for h in range(2):
                        bj = 2 * p + h
                        sl = slice(bj * T, (bj + 1) * T)
                        nc.scalar.activation(out_sb[:, sl], pse[:, h * T:(h + 1) * T],
                                             Act.Relu,
                                             bias=bias_all[:, bj:bj + 1],
                                             scale=S_x[:, bj:bj + 1])
                        nc.vector.bn_stats(ost[:, bj, :], out_sb[:, sl])

            # ---- AllReduce #2 + out affine + store ----
            mv_o = small.tile([C, 2], dt.float32)
            nc.vector.bn_aggr(mv_o[:, :], ost[:, :, :])
            sums2 = small.tile([C, 2], dt.float32)
            nc.vector.tensor_scalar_mul(sums2[:, 0:1], mv_o[:, 0:1], float(NLOC))
            nc.vector.tensor_mul(tmpc[:, :], mv_o[:, 0:1], mv_o[:, 0:1])
            nc.vector.tensor_add(tmpc[:, :], tmpc[:, :], mv_o[:, 1:2])
            nc.vector.tensor_scalar_mul(sums2[:, 1:2], tmpc[:, :], float(NLOC))
            ar2_in = dram.tile([C, 2], dt.float32)
            ar2_out = dram.tile([C, 2], dt.float32, addr_space="Shared")
            nc.sync.dma_start(out=ar2_in[:, :], in_=sums2[:, :])
            nc.gpsimd.collective_compute(
                "AllReduce", Alu.add,
                replica_groups=[list(range(NCORES))],
                ins=[ar2_in[:, :].opt()], outs=[ar2_out[:, :].opt()],
            )
            g2 = small.tile([C, 2], dt.float32)
            nc.sync.dma_start(out=g2[:, :], in_=ar2_out[:, :])
            # reuse g1 slots trick not needed; compute s_o/t_o
            g2w = small.tile([C, 4], dt.float32)
            nc.vector.tensor_copy(g2w[:, 0:2], g2[:, :])
            g1_save = g1
            g1 = g2w
            s_o, t_o = bn_params(0, cols["go"], cols["bto"], "o")
            g1 = g1_save

            for p in range(NCHUNK):
                sl = slice(p * 512, (p + 1) * 512)
                nc.vector.tensor_scalar(out_sb[:, sl], out_sb[:, sl],
                                        s_o[:, :], t_o[:, :],
                                        op0=Alu.mult, op1=Alu.add)
            for bi_ in range(BL):
                nc.gpsimd.dma_start(
                    out=out_d[bi_].rearrange("c j t -> c (j t)"),
                    in_=out_sb[:, bi_ * J * T:(bi_ + 1) * J * T],
                )

    # bacc lowering: wait-splitting onto ldweights, library loads for ap_gather,
    # extended-ISA codegen, nop fusion, register allocation
    nc.compile()
    return nc


_CACHE = {}


def _get_nc():
    if "nc" not in _CACHE:
        _CACHE["nc"] = build_bass()
    return _CACHE["nc"]


def _in_maps(inputs):
    f32 = lambda a: np.ascontiguousarray(np.asarray(a), dtype=np.float32)
    x = f32(inputs["x"])
    wTi = np.ascontiguousarray(f32(inputs["w_int"]).T)
    wTe = np.ascontiguousarray(f32(inputs["w_ext"]).T)
    wf = f32(inputs["w_fus"])
    bfus_col = np.full((C,), float(np.asarray(inputs["b_fus"])), dtype=np.float32)
    common = dict(
        wTi=wTi, wTe=wTe,
        bi=f32(inputs["b_int"]), be=f32(inputs["b_ext"]),
        wfi=np.ascontiguousarray(wf[:C]), wfe=np.ascontiguousarray(wf[C:]),
        bfus=bfus_col,
        gi=f32(inputs["g_int"]), bti=f32(inputs["beta_int"]),
        ge=f32(inputs["g_ext"]), bte=f32(inputs["beta_ext"]),
        go=f32(inputs["g_out"]), bto=f32(inputs["beta_out"]),
    )
    maps = []
    for d in range(NCORES):
        m = dict(common)
        m["x"] = np.ascontiguousarray(
            x[d * BL:(d + 1) * BL].transpose(1, 0, 2, 3).reshape(C, NLOC))
        maps.append(m)
    return maps


def _install_ntff_shim():
    """Register the axon NTFF profile hook (missing antenv.axon_hooks shim)."""
    import sys, types
    if "antenv.axon_hooks" in sys.modules:
        return
    try:
        sys.path.insert(0, "/root/.axon_site")
        from trn_agent_boot.trn_boot import _ntff_profile_via_ctypes
        hook = _ntff_profile_via_ctypes("/opt/axon/libaxon_pjrt.so")
        mod = types.ModuleType("antenv.axon_hooks")
        mod.get_axon_ntff_profile_hook = lambda: hook
        mod.set_axon_ntff_profile_hook = lambda h: None
        sys.modules["antenv.axon_hooks"] = mod
        import concourse.bass_utils as bu
        bu.upload_artifacts = lambda d: d  # no artifact bucket in this container
    except Exception as e:  # pragma: no cover
        print("ntff shim install failed:", e)


def run_spmd(inputs, trace=False):
    if trace:
        _install_ntff_shim()
    nc = _get_nc()
    res = run_bass_kernel_spmd(nc, _in_maps(inputs), list(range(NCORES)), trace=trace)
    return res


def kernel(**inputs):
    res = run_spmd(inputs, trace=False)
    outs = res.results
    out = np.concatenate([outs[d]["out"] for d in range(NCORES)], axis=0)
    path = np.concatenate([outs[d]["path"] for d in range(NCORES)], axis=0)
    idx3 = np.concatenate([outs[d]["idx3"] for d in range(NCORES)], axis=0)
    idx3 = idx3.astype(np.int32, copy=False)
    idx_full = np.broadcast_to(idx3[:, None], (B, C, J, T, 3))
    return out, path, idx_full


# revision 33
# speedup vs baseline: 1.3690x; 1.0477x over previous
"""Trainium2 Bass kernel for nn_ConvOffset2D_nonlocal2 (topk_masking).

Structure exploited:
  matrix[b,j,s,t] = a_ext[b,j,s] + a_int[b,j,t] + b_fus   (rank-1 + broadcast)
so
  out_pre[b,c,j,s] = (a_ext[s]+k)*S_x[c] + D[c],  S_x = sum_t x, D = sum_t a_int[t] x[:,t]
and the masked top-2 over t of matrix[...,s,:] is independent of s (row offset
does not change ranks): per (b,j) it is the top-3 of a_int with two special rows.

Data-parallel over B across 8 cores; sync-BN via two tiny in-kernel AllReduces.
"""

import numpy as np

import concourse.bass as bass
import concourse.mybir as mybir
import concourse.tile as tile
from concourse import bacc, library_config
from concourse.bass_utils import run_bass_kernel_spmd

dt = mybir.dt
Alu = mybir.AluOpType
Act = mybir.ActivationFunctionType

B, C, J, T = 16, 128, 25, 256
NCORES = 8
BL = B // NCORES            # 2 samples per core
NBJ = BL * J                # 50 (b,j) pairs per core
NLOC = BL * J * T           # 12800 free elements per core
NGLOB = float(B * J * T)    # 102400
EPS = 1e-5
NCHUNK = NLOC // 512        # 25 conv chunks of 512


def build_bass():
    nc = bacc.Bacc(None, target_bir_lowering=False)

    # ---- I/O ----
    x_d = nc.declare_dram_parameter("x", [C, NLOC], dt.float32, isOutput=False)
    wTi_d = nc.declare_dram_parameter("wTi", [C, C], dt.float32, isOutput=False)
    wTe_d = nc.declare_dram_parameter("wTe", [C, C], dt.float32, isOutput=False)
    bi_d = nc.declare_dram_parameter("bi", [C], dt.float32, isOutput=False)
    be_d = nc.declare_dram_parameter("be", [C], dt.float32, isOutput=False)
    wfi_d = nc.declare_dram_parameter("wfi", [C], dt.float32, isOutput=False)
    wfe_d = nc.declare_dram_parameter("wfe", [C], dt.float32, isOutput=False)
    bfus_d = nc.declare_dram_parameter("bfus", [C], dt.float32, isOutput=False)
    gi_d = nc.declare_dram_parameter("gi", [C], dt.float32, isOutput=False)
    bti_d = nc.declare_dram_parameter("bti", [C], dt.float32, isOutput=False)
    ge_d = nc.declare_dram_parameter("ge", [C], dt.float32, isOutput=False)
    bte_d = nc.declare_dram_parameter("bte", [C], dt.float32, isOutput=False)
    go_d = nc.declare_dram_parameter("go", [C], dt.float32, isOutput=False)
    bto_d = nc.declare_dram_parameter("bto", [C], dt.float32, isOutput=False)

    out_d = nc.declare_dram_parameter("out", [BL, C, J, T], dt.float32, isOutput=True)
    path_d = nc.declare_dram_parameter("path", [BL, C, J, T, 3], dt.float32, isOutput=True)
    idx3_d = nc.declare_dram_parameter("idx3", [BL, J, T, 3], dt.int32, isOutput=True)

    col = lambda d: d.rearrange("(c one) -> c one", one=1)

    with tile.TileContext(nc) as tc:
        from contextlib import ExitStack

        ctx = ExitStack()
        with ctx:
            consts = ctx.enter_context(tc.tile_pool(name="consts", bufs=1))
            dram = ctx.enter_context(tc.tile_pool(name="dram", bufs=1, space="DRAM"))
            small = ctx.enter_context(tc.tile_pool(name="small", bufs=1))

            # ---- constants to SBUF ----
            wTi = consts.tile([C, C], dt.float32)
            wTe = consts.tile([C, C], dt.float32)
            nc.sync.dma_start(out=wTi[:, :], in_=wTi_d[:, :])
            nc.sync.dma_start(out=wTe[:, :], in_=wTe_d[:, :])
            cols = {}
            for name, d in [("bi", bi_d), ("be", be_d), ("wfi", wfi_d), ("wfe", wfe_d),
                            ("bfus", bfus_d), ("gi", gi_d), ("bti", bti_d), ("ge", ge_d),
                            ("bte", bte_d), ("go", go_d), ("bto", bto_d)]:
                t_ = consts.tile([C, 1], dt.float32, name="c_" + name)
                nc.sync.dma_start(out=t_[:, :], in_=col(d))
                cols[name] = t_
            ones_r = consts.tile([1, C], dt.float32)
            nc.vector.memset(ones_r[:, :], 1.0)
            ones_f = consts.tile([C, C], dt.float32)
            nc.vector.memset(ones_f[:, :], 1.0)

            # ---- x load: (c, b j t) ----
            xpool = ctx.enter_context(tc.tile_pool(name="xpool", bufs=1))
            x_sb = xpool.tile([C, NLOC], dt.float32)
            for p in range(NCHUNK):
                nc.sync.dma_start(
                    out=x_sb[:, p * 512:(p + 1) * 512],
                    in_=x_d[:, p * 512:(p + 1) * 512],
                )

            # ---- conv + relu + bn_stats ----
            ypool = ctx.enter_context(tc.tile_pool(name="ypool", bufs=2))
            y_int = ypool.tile([C, NLOC], dt.float32, tag="big")
            y_ext = ypool.tile([C, NLOC], dt.float32, tag="big")
            stats = ctx.enter_context(tc.tile_pool(name="stats", bufs=1))
            st_i = stats.tile([C, NCHUNK, 6], dt.float32)
            st_e = stats.tile([C, NCHUNK, 6], dt.float32)

            with tc.tile_pool(name="convps", bufs=4, space="PSUM") as convps:
                for (wT, bcol, y_sb, st) in [(wTi, cols["bi"], y_int, st_i),
                                             (wTe, cols["be"], y_ext, st_e)]:
                    for p in range(NCHUNK):
                        sl = slice(p * 512, (p + 1) * 512)
                        ps = convps.tile([C, 512], dt.float32, tag="cps")
                        nc.tensor.matmul(ps[:, :], wT[:, :], x_sb[:, sl],
                                         start=True, stop=True)
                        nc.scalar.activation(y_sb[:, sl], ps[:, :], Act.Relu,
                                             bias=bcol[:, :], scale=1.0)
                        nc.vector.bn_stats(st[:, p, :], y_sb[:, sl])

            # S_x[c, bj] = sum_t x  (emitted here; scheduler overlaps)
            S_x = small.tile([C, NBJ], dt.float32)
            nc.vector.tensor_reduce(S_x[:, :], x_sb.rearrange("c (bj t) -> c bj t", t=T),
                                    axis=mybir.AxisListType.X, op=Alu.add)

            # ---- local sums + AllReduce #1 ----
            mv_i = small.tile([C, 2], dt.float32)
            mv_e = small.tile([C, 2], dt.float32)
            nc.vector.bn_aggr(mv_i[:, :], st_i[:, :, :])
            nc.vector.bn_aggr(mv_e[:, :], st_e[:, :, :])
            sums4 = small.tile([C, 4], dt.float32)
            tmpc = small.tile([C, 1], dt.float32)
            for k, mv in [(0, mv_i), (2, mv_e)]:
                nc.vector.tensor_scalar_mul(sums4[:, k:k + 1], mv[:, 0:1], float(NLOC))
                nc.vector.tensor_mul(tmpc[:, :], mv[:, 0:1], mv[:, 0:1])
                nc.vector.tensor_add(tmpc[:, :], tmpc[:, :], mv[:, 1:2])
                nc.vector.tensor_scalar_mul(sums4[:, k + 1:k + 2], tmpc[:, :], float(NLOC))

            ar1_in = dram.tile([C, 4], dt.float32)
            ar1_out = dram.tile([C, 4], dt.float32, addr_space="Shared")
            nc.sync.dma_start(out=ar1_in[:, :], in_=sums4[:, :])
            nc.gpsimd.collective_compute(
                "AllReduce", Alu.add,
                replica_groups=[list(range(NCORES))],
                ins=[ar1_in[:, :].opt()], outs=[ar1_out[:, :].opt()],
            )
            g1 = small.tile([C, 4], dt.float32)
            nc.sync.dma_start(out=g1[:, :], in_=ar1_out[:, :])

            # ---- global affine params per branch ----
            def bn_params(goff, gcol, btcol, tag):
                mean = small.tile([C, 1], dt.float32, name="mean_" + tag)
                ex2 = small.tile([C, 1], dt.float32, name="ex2_" + tag)
                var = small.tile([C, 1], dt.float32, name="var_" + tag)
                nc.vector.tensor_scalar_mul(mean[:, :], g1[:, goff:goff + 1], 1.0 / NGLOB)
                nc.vector.tensor_scalar_mul(ex2[:, :], g1[:, goff + 1:goff + 2], 1.0 / NGLOB)
                nc.vector.tensor_mul(var[:, :], mean[:, :], mean[:, :])
                nc.vector.tensor_sub(var[:, :], ex2[:, :], var[:, :])
                # rsqrt(var+eps) with 2 Newton steps (ACT sqrt table is low precision)
                sd = small.tile([C, 1], dt.float32, name="sd_" + tag)
                veps = small.tile([C, 1], dt.float32, name="veps_" + tag)
                nc.vector.tensor_scalar_add(veps[:, :], var[:, :], EPS)
                nc.scalar.activation(sd[:, :], veps[:, :], Act.Sqrt)
                r = small.tile([C, 1], dt.float32, name="r_" + tag)
                nc.vector.reciprocal(r[:, :], sd[:, :])
                t1 = small.tile([C, 1], dt.float32, name="t1_" + tag)
                for _ in range(2):
                    nc.vector.tensor_mul(t1[:, :], r[:, :], r[:, :])
                    nc.vector.tensor_mul(t1[:, :], t1[:, :], veps[:, :])
                    nc.vector.tensor_scalar(t1[:, :], t1[:, :], -0.5, 1.5,
                                            op0=Alu.mult, op1=Alu.add)
                    nc.vector.tensor_mul(r[:, :], r[:, :], t1[:, :])
                s_ = small.tile([C, 1], dt.float32, name="s_" + tag)
                t_ = small.tile([C, 1], dt.float32, name="t_" + tag)
                nc.vector.tensor_mul(s_[:, :], r[:, :], gcol[:, :])
                nc.vector.tensor_mul(t1[:, :], mean[:, :], s_[:, :])
                nc.vector.tensor_sub(t_[:, :], btcol[:, :], t1[:, :])
                return s_, t_

            s_i, t_i = bn_params(0, cols["gi"], cols["bti"], "i")
            s_e, t_e = bn_params(2, cols["ge"], cols["bte"], "e")
            u_i = small.tile([C, 1], dt.float32)
            u_e = small.tile([C, 1], dt.float32)
            nc.vector.tensor_mul(u_i[:, :], s_i[:, :], cols["wfi"][:, :])
            nc.vector.tensor_mul(u_e[:, :], s_e[:, :], cols["wfe"][:, :])

            # k0 = sum_c(wfi*t_i) + sum_c(wfe*t_e) + b_fus, broadcast to all partitions
            zc = small.tile([C, 1], dt.float32)
            z2 = small.tile([C, 1], dt.float32)
            nc.vector.tensor_mul(zc[:, :], t_i[:, :], cols["wfi"][:, :])
            nc.vector.tensor_mul(z2[:, :], t_e[:, :], cols["wfe"][:, :])
            nc.vector.tensor_add(zc[:, :], zc[:, :], z2[:, :])
            k0 = small.tile([C, 1], dt.float32)
            with tc.tile_pool(name="kps", bufs=1, space="PSUM") as kps:
                psk = kps.tile([C, 1], dt.float32)
                nc.tensor.matmul(psk[:, :], ones_f[:, :], zc[:, :], start=True, stop=True)
                nc.vector.tensor_add(k0[:, :], psk[:, :], cols["bfus"][:, :])

            # ---- a-dots: a_rows[p, h*256+s] for bj=2p+h ----
            # psum rows land on partition 0; engines cannot write partition p!=0/32/64/96,
            # so each row bounces through DRAM and the (25,512) layout comes back by DMA.
            a_i = small.tile([J, 512], dt.float32)
            a_e = small.tile([J, 512], dt.float32)
            a0d_i = dram.tile([J, 512], dt.float32)
            a0d_e = dram.tile([J, 512], dt.float32)
            arow = ctx.enter_context(tc.tile_pool(name="arow", bufs=2))
            with tc.tile_pool(name="aps", bufs=4, space="PSUM") as aps:
                for p in range(NCHUNK):
                    sl = slice(p * 512, (p + 1) * 512)
                    psa = aps.tile([1, 512], dt.float32, tag="aps")
                    nc.tensor.matmul(psa[:, :], u_i[:, :], y_int[:, sl],
                                     start=True, stop=True)
                    ta = arow.tile([1, 512], dt.float32, tag="ar")
                    nc.scalar.copy(ta[:, :], psa[:, :])
                    nc.sync.dma_start(out=a0d_i[p:p + 1, :], in_=ta[:, :])
                    psb = aps.tile([1, 512], dt.float32, tag="aps")
                    nc.tensor.matmul(psb[:, :], u_e[:, :], y_ext[:, sl],
                                     start=True, stop=True)
                    tb = arow.tile([1, 512], dt.float32, tag="ar")
                    nc.vector.tensor_copy(tb[:, :], psb[:, :])
                    nc.sync.dma_start(out=a0d_e[p:p + 1, :], in_=tb[:, :])
            nc.sync.dma_start(out=a_i[:, :], in_=a0d_i[:, :])
            nc.sync.dma_start(out=a_e[:, :], in_=a0d_e[:, :])

            # ---- top-3 + sorted index triples (per half h: bj = 2p+h) ----
            idxp = ctx.enter_context(tc.tile_pool(name="idxp", bufs=1))
            iota = idxp.tile([J, T], dt.int32)
            nc.gpsimd.iota(iota[:, :], pattern=[[1, T]], base=0, channel_multiplier=0)
            iota_f = idxp.tile([J, T], dt.float32)
            nc.vector.tensor_copy(iota_f[:, :], iota[:, :])
            jo = idxp.tile([J, 1], dt.int32)
            nc.gpsimd.iota(jo[:, :], pattern=[[0, 1]], base=0, channel_multiplier=512)
            jof = idxp.tile([J, 1], dt.float32)
            nc.vector.tensor_copy(jof[:, :], jo[:, :])
            joff = []
            msk = idxp.tile([J, 1], dt.float32)
            for h in range(2):
                jt = idxp.tile([J, 1], dt.float32, name=f"joff{h}")
                nc.vector.tensor_scalar_add(jt[:, :], jof[:, :], float(256 * h))
                # rows with bj = 2p+h >= 25 wrap to sample b=1: subtract 6400
                lo = 13 - h
                nc.vector.tensor_scalar(msk[:, :], jof[:, :], float(512 * lo), None,
                                        op0=Alu.is_ge)
                nc.vector.scalar_tensor_tensor(jt[:, :], msk[:, :], -6400.0, jt[:, :],
                                               op0=Alu.mult, op1=Alu.add)
                joff.append(jt)

            idx_i32 = idxp.tile([J, 2 * T * 3], dt.int32)     # (25, 1536)
            gidx16 = idxp.tile([J, 2 * T * 3], dt.int16)
            mx8 = idxp.tile([J, 8], dt.float32)
            ix8 = idxp.tile([J, 8], dt.uint32)
            qb = idxp.tile([J, T], dt.float32)
            em = idxp.tile([J, T], dt.int32)
            e2 = idxp.tile([J, T], dt.int32)
            planes = ctx.enter_context(tc.tile_pool(name="planes", bufs=3))
            ccols = ctx.enter_context(tc.tile_pool(name="ccols", bufs=8))

            for h in range(2):
                half = slice(h * T, (h + 1) * T)
                nc.vector.max(mx8[:, :], a_i[:, half])
                nc.vector.max_index(ix8[:, :], mx8[:, :], a_i[:, half])
                i1 = ccols.tile([J, 1], dt.float32, tag="cc")
                i2 = ccols.tile([J, 1], dt.float32, tag="cc")
                i3 = ccols.tile([J, 1], dt.float32, tag="cc")
                nc.vector.tensor_copy(i1[:, :], ix8[:, 0:1])
                nc.vector.tensor_copy(i2[:, :], ix8[:, 1:2])
                nc.vector.tensor_copy(i3[:, :], ix8[:, 2:3])
                m1 = ccols.tile([J, 1], dt.float32, tag="cc")
                m2 = ccols.tile([J, 1], dt.float32, tag="cc")
                nc.vector.tensor_tensor(m1[:, :], i1[:, :], i2[:, :], op=Alu.min)
                nc.vector.tensor_tensor(m2[:, :], i1[:, :], i2[:, :], op=Alu.max)
                q0 = ccols.tile([J, 1], dt.float32, tag="cc")
                q2 = ccols.tile([J, 1], dt.float32, tag="cc")
                q1 = ccols.tile([J, 1], dt.float32, tag="cc")
                nc.vector.tensor_tensor(q0[:, :], m1[:, :], i3[:, :], op=Alu.min)
                nc.vector.tensor_tensor(q2[:, :], m2[:, :], i3[:, :], op=Alu.max)
                nc.vector.tensor_add(q1[:, :], i1[:, :], i2[:, :])
                nc.vector.tensor_add(q1[:, :], q1[:, :], i3[:, :])
                nc.vector.tensor_sub(q1[:, :], q1[:, :], q0[:, :])
                nc.vector.tensor_sub(q1[:, :], q1[:, :], q2[:, :])

                # special-row mask: s==i1 or s==i2
                nc.vector.tensor_scalar(em[:, :], iota_f[:, :], i1[:, :], None, op0=Alu.is_equal)
                nc.vector.tensor_scalar(e2[:, :], iota_f[:, :], i2[:, :], None, op0=Alu.is_equal)
                nc.vector.tensor_add(em[:, :], em[:, :], e2[:, :])

                for l, qq in enumerate([q0, q1, q2]):
                    pl = planes.tile([J, T], dt.float32, tag="pl")
                    if l == 0:
                        nc.vector.tensor_scalar(pl[:, :], iota_f[:, :], m1[:, :], None, op0=Alu.min)
                    elif l == 2:
                        nc.vector.tensor_scalar(pl[:, :], iota_f[:, :], m2[:, :], None, op0=Alu.max)
                    else:
                        nc.vector.tensor_scalar(pl[:, :], iota_f[:, :], m1[:, :], m2[:, :],
                                                op0=Alu.max, op1=Alu.min)
                    # qb = broadcast(qq); overwrite special rows
                    nc.vector.tensor_scalar(qb[:, :], iota_f[:, :], 0.0, qq[:, :],
                                            op0=Alu.mult, op1=Alu.add)
                    nc.vector.copy_predicated(pl[:, :], em[:, :], qb[:, :])
                    # interleave into (s,l) layout: idx3 output (int32 cast)
                    dst = bass.AP(tensor=idx_i32.tensor, offset=idx_i32.offset + h * T * 3 + l,
                                  ap=[list(idx_i32.ap[0]), [3, T]])
                    nc.vector.tensor_copy(dst, pl[:, :])
                    # gather index with +j*256 offset (int16 cast)
                    gdst = bass.AP(tensor=gidx16.tensor, offset=gidx16.offset + h * T * 3 + l,
                                   ap=[list(gidx16.ap[0]), [3, T]])
                    nc.vector.tensor_scalar_add(gdst, pl[:, :], joff[h][:, :])

            # idx3 output DMA: rows are bj pairs -> flat (b j s l)
            idx3_flat = bass.AP(tensor=idx3_d, offset=0, ap=[[1536, J], [1, 1536]])
            nc.sync.dma_start(out=idx3_flat, in_=idx_i32[:, :])

            # gather-index wrap for ap_gather's per-core layout:
            # glist (flat, per b) -> widx[p, w] = glist[w*16 + p%16].
            # Done via the xbar transpose engine: read scratch as (1200, 128)
            # rows overlapping at stride 16; transposed rows 0:16 are the wrap.
            # Then replicate rows 0:16 to all 8 16-partition blocks via DRAM.
            scratch = dram.tile([J + 1, 1536], dt.int16)  # +1 row pad for overlap reads
            nc.sync.dma_start(out=scratch[0:J, :], in_=gidx16[:, :])
            widx = idxp.tile([C, 2, 1200], dt.int16)
            scratch2 = dram.tile([BL, 32, 1200], dt.int16)
            for b_ in range(BL):
                src = bass.AP(tensor=scratch.tensor,
                              offset=scratch.offset + b_ * 19200,
                              ap=[[16, 1200], [1, 128]])
                nc.sync.dma_start_transpose(out=widx[:, b_, :], in_=src)
                nc.sync.dma_start(out=scratch2[b_, 0:16, :], in_=widx[0:16, b_, :])
                # duplicate to 32 rows in DRAM, then fill all four 32-row groups
                nc.sync.dma_start(out=scratch2[b_, 16:32, :], in_=scratch2[b_, 0:16, :])
                for m in range(4):
                    nc.sync.dma_start(out=widx[32 * m:32 * (m + 1), b_, :],
                                      in_=scratch2[b_, :, :])

            # ---- gather path values + write out ----
            # Strided HBM writes run ~20 GB/s per DMA ring and rings drain
            # FIFO, so split every chunk's write across both HWDGE rings
            # (sync stays free for the small loop DMAs) + the SWDGE ring.
            gpool = ctx.enter_context(tc.tile_pool(name="gpool", bufs=2))
            path_flat = path_d.rearrange("b c j t l -> b c (j t l)")
            ci = 0
            for b_ in range(BL):
                for jc in range(0, J, 2):
                    nj = min(2, J - jc)
                    nidx = nj * 768
                    gout = gpool.tile([C, 1536], dt.float32, tag="gout")
                    nc.gpsimd.ap_gather(
                        gout[:, :nidx],
                        x_sb[:, b_ * J * T:(b_ + 1) * J * T],
                        widx[:, b_, jc * 48:jc * 48 + nidx // 16],
                        channels=C, num_elems=J * T, d=1, num_idxs=nidx,
                    )
                    half = nidx // 2
                    e1, e2 = ((nc.scalar, nc.gpsimd) if ci % 2 == 0
                              else (nc.gpsimd, nc.scalar))
                    ci += 1
                    e1.dma_start(
                        out=path_flat[b_, :, jc * 768:jc * 768 + half],
                        in_=gout[:, :half],
                    )
                    e2.dma_start(
                        out=path_flat[b_, :, jc * 768 + half:jc * 768 + nidx],
                        in_=gout[:, half:nidx],
                    )

            # ---- D[c,bj] = sum_t a_int[t] x[c,t] ; bias_all = k0*S_x + D ----
            Dall = small.tile([C, NBJ], dt.float32)
            bias_all = small.tile([C, NBJ], dt.float32)
            djp = ctx.enter_context(tc.tile_pool(name="djp", bufs=2))
            rpool = ctx.enter_context(tc.tile_pool(name="rpool", bufs=3))
            with tc.tile_pool(name="bps1", bufs=2, space="PSUM") as bps1:
                for p in range(NCHUNK):
                    rhs = rpool.tile([1, 512], dt.float32, tag="rhs")
                    nc.sync.dma_start(out=rhs[:, :], in_=a0d_i[p:p + 1, :])
                    psd = bps1.tile([C, 512], dt.float32, tag="bps1")
                    nc.tensor.matmul(psd[:, :], ones_r[:, :].bitcast(dt.float32r),
                                     rhs[:, :].bitcast(dt.float32r),
                                     start=True, stop=True)
                    for h in range(2):
                        bj = 2 * p + h
                        dj = djp.tile([C, T], dt.float32, tag="dj")
                        nc.vector.scalar_tensor_tensor(
                            dj[:, :], x_sb[:, bj * T:(bj + 1) * T], 1.0,
                            psd[:, h * T:(h + 1) * T],
                            op0=Alu.mult, op1=Alu.mult,
                            accum_out=Dall[:, bj:bj + 1])
            nc.vector.scalar_tensor_tensor(bias_all[:, :], S_x[:, :], k0[:, :],
                                           Dall[:, :], op0=Alu.mult, op1=Alu.add)

            # ---- out_pre -> relu -> bn_stats ----
            out_sb = ypool.tile([C, NLOC], dt.float32, tag="big")
            ost = stats.tile([C, NBJ, 6], dt.float32)
            with tc.tile_pool(name="bps2", bufs=2, space="PSUM") as bps2:
                for p in range(NCHUNK):
                    rhs = rpool.tile([1, 512], dt.float32, tag="rhs")
                    nc.sync.dma_start(out=rhs[:, :], in_=a0d_e[p:p + 1, :])
                    pse = bps2.tile([C, 512], dt.float32, tag="bps2")
                    nc.tensor.matmul(pse[:, :], ones_r[:, :].bitcast(dt.float32r),
                                     rhs[:, :].bitcast(dt.float32r),
                                     start=True, stop=True)
                    for h in range(2):
                        bj = 2 * p + h
                        sl = slice(bj * T, (bj + 1) * T)
                        nc.scalar.activation(out_sb[:, sl], pse[:, h * T:(h + 1) * T],
                                             Act.Relu,
                                             bias=bias_all[:, bj:bj + 1],
                                             scale=S_x[:, bj:bj + 1])
                        nc.vector.bn_stats(ost[:, bj, :], out_sb[:, sl])

            # ---- AllReduce #2 + out affine + store ----
            mv_o = small.tile([C, 2], dt.float32)
            nc.vector.bn_aggr(mv_o[:, :], ost[:, :, :])
            sums2 = small.tile([C, 2], dt.float32)
            nc.vector.tensor_scalar_mul(sums2[:, 0:1], mv_o[:, 0:1], float(NLOC))
            nc.vector.tensor_mul(tmpc[:, :], mv_o[:, 0:1], mv_o[:, 0:1])
            nc.vector.tensor_add(tmpc[:, :], tmpc[:, :], mv_o[:, 1:2])
            nc.vector.tensor_scalar_mul(sums2[:, 1:2], tmpc[:, :], float(NLOC))
            ar2_in = dram.tile([C, 2], dt.float32)
            ar2_out = dram.tile([C, 2], dt.float32, addr_space="Shared")
            nc.sync.dma_start(out=ar2_in[:, :], in_=sums2[:, :])
            nc.gpsimd.collective_compute(
                "AllReduce", Alu.add,
                replica_groups=[list(range(NCORES))],
                ins=[ar2_in[:, :].opt()], outs=[ar2_out[:, :].opt()],
            )
            g2 = small.tile([C, 2], dt.float32)
            nc.sync.dma_start(out=g2[:, :], in_=ar2_out[:, :])
            # reuse g1 slots trick not needed; compute s_o/t_o
            g2w = small.tile([C, 4], dt.float32)
            nc.vector.tensor_copy(g2w[:, 0:2], g2[:, :])
            g1_save = g1
            g1 = g2w
            s_o, t_o = bn_params(0, cols["go"], cols["bto"], "o")
            g1 = g1_save

            for p in range(NCHUNK):
                sl = slice(p * 512, (p + 1) * 512)
                nc.vector.tensor_scalar(out_sb[:, sl], out_sb[:, sl],
                                        s_o[:, :], t_o[:, :],
                                        op0=Alu.mult, op1=Alu.add)
            outengs = [nc.scalar, nc.gpsimd, nc.sync, nc.scalar]
            for bi_ in range(BL):
                of = out_d[bi_].rearrange("c j t -> c (j t)")
                hj = J * T // 2
                for hh in range(2):
                    outengs[bi_ * 2 + hh].dma_start(
                        out=of[:, hh * hj:(hh + 1) * hj],
                        in_=out_sb[:, bi_ * J * T + hh * hj:bi_ * J * T + (hh + 1) * hj],
                    )

    # bacc lowering: wait-splitting onto ldweights, library loads for ap_gather,
    # extended-ISA codegen, nop fusion, register allocation
    nc.compile()
    return nc


_CACHE = {}


def _get_nc():
    if "nc" not in _CACHE:
        _CACHE["nc"] = build_bass()
    return _CACHE["nc"]


def _in_maps(inputs):
    f32 = lambda a: np.ascontiguousarray(np.asarray(a), dtype=np.float32)
    x = f32(inputs["x"])
    wTi = np.ascontiguousarray(f32(inputs["w_int"]).T)
    wTe = np.ascontiguousarray(f32(inputs["w_ext"]).T)
    wf = f32(inputs["w_fus"])
    bfus_col = np.full((C,), float(np.asarray(inputs["b_fus"])), dtype=np.float32)
    common = dict(
        wTi=wTi, wTe=wTe,
        bi=f32(inputs["b_int"]), be=f32(inputs["b_ext"]),
        wfi=np.ascontiguousarray(wf[:C]), wfe=np.ascontiguousarray(wf[C:]),
        bfus=bfus_col,
        gi=f32(inputs["g_int"]), bti=f32(inputs["beta_int"]),
        ge=f32(inputs["g_ext"]), bte=f32(inputs["beta_ext"]),
        go=f32(inputs["g_out"]), bto=f32(inputs["beta_out"]),
    )
    maps = []
    for d in range(NCORES):
        m = dict(common)
        m["x"] = np.ascontiguousarray(
            x[d * BL:(d + 1) * BL].transpose(1, 0, 2, 3).reshape(C, NLOC))
        maps.append(m)
    return maps


def _install_ntff_shim():
    """Register the axon NTFF profile hook (missing antenv.axon_hooks shim)."""
    import sys, types
    if "antenv.axon_hooks" in sys.modules:
        return
    try:
        sys.path.insert(0, "/root/.axon_site")
        from trn_agent_boot.trn_boot import _ntff_profile_via_ctypes
        hook = _ntff_profile_via_ctypes("/opt/axon/libaxon_pjrt.so")
        mod = types.ModuleType("antenv.axon_hooks")
        mod.get_axon_ntff_profile_hook = lambda: hook
        mod.set_axon_ntff_profile_hook = lambda h: None
        sys.modules["antenv.axon_hooks"] = mod
        import concourse.bass_utils as bu
        bu.upload_artifacts = lambda d: d  # no artifact bucket in this container
    except Exception as e:  # pragma: no cover
        print("ntff shim install failed:", e)


def run_spmd(inputs, trace=False):
    if trace:
        _install_ntff_shim()
    nc = _get_nc()
    res = run_bass_kernel_spmd(nc, _in_maps(inputs), list(range(NCORES)), trace=trace)
    return res


def kernel(**inputs):
    res = run_spmd(inputs, trace=False)
    outs = res.results
    out = np.concatenate([outs[d]["out"] for d in range(NCORES)], axis=0)
    path = np.concatenate([outs[d]["path"] for d in range(NCORES)], axis=0)
    idx3 = np.concatenate([outs[d]["idx3"] for d in range(NCORES)], axis=0)
    idx3 = idx3.astype(np.int32, copy=False)
    idx_full = np.broadcast_to(idx3[:, None], (B, C, J, T, 3))
    return out, path, idx_full
